# revision 20
# baseline (speedup 1.0000x reference)
"""minLSTM (2-layer, B=4, S=4096, D=1024) on 8 Trainium2 NeuronCores.

Sharding: core k -> (batch b = k//2, channel half h = k%2).
Each core computes all 4096 timesteps for its batch and its 512 channels:
  - gates via PE matmuls in bf16 (lhsT = W^T shard, rhs = x^T), laid out
    (gate-channel partition x token free) so the recurrence layout is native,
  - normalized gates f' = sig(f)/(sig(f)+sig(i)) via ACT sigmoids + one DVE
    reciprocal, with the cheap elementwise ops (ssum, g-max, h-mult) offloaded
    to the otherwise-idle Pool engine,
  - g = max(cell+b+0.5, sig(cell+b)),
  - b~ = (f'-1)*g via one fused scalar_tensor_tensor, then the recurrence
    c_t = f'*c_{t-1} + (1-f')*g_t as tensor_tensor_scan(mult, subtract),
  - h = sig(o) * c.
x-tile loads are prefetched 3 (block,layer) units ahead so they never queue
behind h-stores on the in-order SP DMA queue. Between the two layers,
channel-half pairs exchange h1 (bf16) via pairwise AllGather collectives.

Self-contained: hardcodes shapes; only imports the system concourse repo.
"""
import sys

if '/opt/trn_rl_repo' not in sys.path:
    sys.path.insert(0, '/opt/trn_rl_repo')

import numpy as np

B, S, D = 4, 4096, 1024
NCORES = 8
HALF = D // 2           # channels per core: 512
NCHUNK = HALF // 128    # 4 partition chunks of 128 channels
NKT = D // 128          # 8 contraction k-tiles
TBLK = 512              # token block
NBLK = S // TBLK        # 8 token blocks
GCH = 4 * HALF          # gate channels per core: 2048
PF = 3                  # x-tile prefetch depth in (layer, block) units

_CACHE = {}


def _split_multi_waits(nc):
    """This walrus build rejects >1 sync wait per instruction. Hoist extra
    waits onto same-engine NoOps inserted just before; engine-queue program
    order makes this semantically identical."""
    from concourse import mybir
    n = 0
    for fn in nc.m.functions:
        for blk in fn.blocks:
            insts = list(blk.instructions)
            new = []
            changed = False
            for inst in insts:
                si = inst.sync_info
                ow = list(si.on_wait) if si is not None and si.on_wait else []
                if len(ow) > 1:
                    changed = True
                    for w in ow[:-1]:
                        n += 1
                        nop = mybir.InstNoOp(name=f"I-wsplit-{n}", ins=[], outs=[])
                        nop.engine = inst.engine
                        nop.sync_info = mybir.SyncInfo(on_wait=[w], on_update=[])
                        new.append(nop)
                    si.on_wait = [ow[-1]]
                new.append(inst)
            if changed:
                blk.instructions = new
    return n


def _build_nc(mm_mode="bf16"):
    import concourse.bass as bass
    import concourse.mybir as mybir
    import concourse.tile as tile

    f32 = mybir.dt.float32
    fmm = {"f32r": mybir.dt.float32r, "f32": f32,
           "bf16": mybir.dt.bfloat16}[mm_mode]
    fh1 = mybir.dt.bfloat16 if mm_mode == "bf16" else f32
    AF = mybir.ActivationFunctionType
    ALU = mybir.AluOpType

    nc = bass.Bass("TRN2", target_bir_lowering=False, debug=False,
                   num_devices=NCORES)

    xT_d = nc.dram_tensor("xT", [D, S], fmm, kind="ExternalInput").ap()
    w_d = [nc.dram_tensor(f"w{l}t", [D, GCH], fmm, kind="ExternalInput").ap()
           for l in range(2)]
    ba_d = [nc.dram_tensor(f"b{l}a", [128, 16], f32, kind="ExternalInput").ap()
            for l in range(2)]
    bc_d = [nc.dram_tensor(f"b{l}c", [128, 4], f32, kind="ExternalInput").ap()
            for l in range(2)]
    cp_d = [nc.dram_tensor(f"cp{l}", [128, 4], f32, kind="ExternalInput").ap()
            for l in range(2)]
    h2t_d = nc.dram_tensor("h2t", [HALF, S], f32, kind="ExternalOutput").ap()

    with tile.TileContext(nc) as tc:
        with tc.tile_pool(name="wp", bufs=1) as wp, \
             tc.tile_pool(name="xkp", bufs=PF + 1) as xkp, \
             tc.tile_pool(name="gp", bufs=2) as gp, \
             tc.tile_pool(name="cgp", bufs=3) as cgp, \
             tc.tile_pool(name="cp", bufs=1) as cpool, \
             tc.tile_pool(name="psum", bufs=8, space="PSUM") as psum, \
             tc.tile_pool(name="dstage", bufs=4, space="DRAM") as dstage, \
             tc.tile_pool(name="dfull", bufs=8, space="DRAM") as dfull:

            # h1 gathered blocks must persist through layer 2: 8 live tiles
            h1f = [dfull.tile([D, TBLK], fh1, tag="h1f", name=f"h1f{t}")
                   for t in range(NBLK)]

            # Weight layout (host side): gate-channel index ct = j*4 + q so a
            # chunk j's four gate slices are one contiguous [128,512] span.
            # Layer-0 weights stream per (chunk, k) on the SP queue right
            # behind the first x block; layer-1 weights go on the ACT hwdge
            # queue so they never delay layer-0's pipeline.
            w_ks = {}
            ba = {}
            bc = {}
            cp = {}
            for l in range(2):
                ba[l] = cpool.tile([128, 16], f32, tag=f"ba{l}", name=f"ba{l}")
                bc[l] = cpool.tile([128, 4], f32, tag=f"bc{l}", name=f"bc{l}")
                cp[l] = cpool.tile([128, 4], f32, tag=f"cp{l}", name=f"cp{l}")
                w_ks[l] = [wp.tile([128, GCH], fmm, tag=f"Wk{l}_{k}",
                                   name=f"w{l}_{k}") for k in range(NKT)]

            units = [(l, t) for l in range(2) for t in range(NBLK)]
            xk_tiles = {}

            def load_unit(u):
                l, t = units[u]
                xk_ks = []
                for k in range(NKT):
                    xkt = xkp.tile([128, TBLK], fmm, tag=f"xk{k}",
                                   name=f"xk{l}_{t}_{k}")
                    if l == 0:
                        src = xT_d[k * 128:(k + 1) * 128,
                                   t * TBLK:(t + 1) * TBLK]
                    else:
                        src = h1f[t][k * 128:(k + 1) * 128, :]
                    nc.sync.dma_start(
                        xkt[:],
                        src if src.dtype == fmm else src.bitcast(fmm))
                    xk_ks.append(xkt)
                xk_tiles[u] = xk_ks

            # Startup order: first x block and the weight k-tiles race to
            # feed the first chunk (which consumes k-outer, below); biases
            # and everything else follow. Layer-1 weights ride the ACT hwdge
            # queue so they never delay layer 0.
            nc.sync.dma_start(w_ks[0][0][:], w_d[0][0:128, :])
            load_unit(0)
            for k in range(1, NKT):
                nc.sync.dma_start(w_ks[0][k][:],
                                  w_d[0][k * 128:(k + 1) * 128, :])
            for l in range(2):
                nc.sync.dma_start(ba[l][:], ba_d[l][:])
                nc.sync.dma_start(bc[l][:], bc_d[l][:])
                nc.sync.dma_start(cp[l][:], cp_d[l][:])
            for u in range(1, min(PF, len(units))):
                load_unit(u)
            for k in range(NKT):
                nc.scalar.dma_start(w_ks[1][k][:],
                                    w_d[1][k * 128:(k + 1) * 128, :])

            def act_recip(out_ap, in_ap):
                """ACT-table reciprocal (bass blocks AF.Reciprocal in
                activation(); measured 1.2e-5 max rel err on [9e-5, 2],
                far inside this kernel's tolerance)."""
                eng = nc.scalar
                ins_ = [eng.lower_ap(in_ap)]
                for argv in (0.0, 1.0, 0.0):  # bias, scale, alpha imms
                    ins_.append(mybir.ImmediateValue(dtype=f32, value=argv))
                eng.add_instruction(mybir.InstActivation(
                    name=nc.get_next_instruction_name(),
                    func=AF.Reciprocal,
                    ins=ins_,
                    outs=[eng.lower_ap(out_ap)]))

            carry = {0: [None] * NCHUNK, 1: [None] * NCHUNK}
            for u, (l, t) in enumerate(units):
                if u + PF < len(units):
                    load_unit(u + PF)
                xk_ks = xk_tiles.pop(u)

                if l == 0:
                    h1own = dstage.tile([HALF, TBLK], fh1, tag="h1own",
                                        name=f"h1own{t}")

                def col(j, qi_):
                    return ba[l][:, j * 4 + qi_:j * 4 + qi_ + 1]

                # Per half-block (2 chunks): matmuls + sigmoid-table ACT ops
                # + Pool ssum, then the 2 ACT reciprocals batched so only
                # two table swaps happen per half-block, then the DVE chain.
                # The reciprocal runs on ACT (which has slack), leaving the
                # DVE with only g/btn/scan/h — every engine is now well
                # under the PE's 27.6us/block.
                for jp in range(NCHUNK // 2):
                    tiles = {}
                    for j in (2 * jp, 2 * jp + 1):
                        ps = {}
                        for q in ("i", "f", "o", "cell"):
                            ps[q] = psum.tile([128, TBLK], f32, tag="ps",
                                              name=f"ps_{q}{l}_{t}_{j}")
                        # k-inner (8 MMs per PSUM group) avoids the HAM
                        # psum-cycling throttle; the very first chunk goes
                        # k-outer so the PE starts while weight k-tiles are
                        # still arriving (HAM is cold then anyway).
                        if u == 0 and j == 0:
                            mm_order = [(k, qi) for k in range(NKT)
                                        for qi in range(4)]
                        else:
                            mm_order = [(k, qi) for qi in range(4)
                                        for k in range(NKT)]
                        qnames = ("i", "f", "o", "cell")
                        for k, qi in mm_order:
                            ct = j * 4 + qi
                            nc.tensor.matmul(
                                ps[qnames[qi]][:],
                                w_ks[l][k][:, ct * 128:(ct + 1) * 128],
                                xk_ks[k][:],
                                start=(k == 0), stop=(k == NKT - 1))

                        sf = gp.tile([128, TBLK], f32, tag="sf", bufs=4,
                                     name=f"sf{l}{t}{j}")
                        nc.scalar.activation(sf[:], ps["f"][:], AF.Sigmoid,
                                             bias=col(j, 1))
                        si = gp.tile([128, TBLK], f32, tag="si", bufs=3,
                                     name=f"si{l}{t}{j}")
                        nc.scalar.activation(si[:], ps["i"][:], AF.Sigmoid,
                                             bias=col(j, 0))
                        sg = gp.tile([128, TBLK], f32, tag="sg", bufs=4,
                                     name=f"sg{l}{t}{j}")
                        nc.scalar.activation(sg[:], ps["cell"][:], AF.Sigmoid,
                                             bias=col(j, 3))
                        cp5 = gp.tile([128, TBLK], f32, tag="cp5", bufs=4,
                                      name=f"cq{l}{t}{j}")
                        nc.scalar.activation(cp5[:], ps["cell"][:],
                                             AF.Identity,
                                             bias=bc[l][:, j:j + 1])
                        so = gp.tile([128, TBLK], f32, tag="so", bufs=5,
                                     name=f"so{l}{t}{j}")
                        nc.scalar.activation(so[:], ps["o"][:], AF.Sigmoid,
                                             bias=col(j, 2))
                        ssum = gp.tile([128, TBLK], f32, tag="ssum", bufs=3,
                                       name=f"ss{l}{t}{j}")
                        nc.gpsimd.tensor_tensor(ssum[:], sf[:], si[:],
                                                ALU.add)
                        tiles[j] = (sf, sg, cp5, so, ssum)

                    rr = {}
                    last_hb = (u == len(units) - 1 and jp == NCHUNK // 2 - 1)
                    for j in (2 * jp, 2 * jp + 1):
                        r = gp.tile([128, TBLK], f32, tag="r", bufs=3,
                                    name=f"r{l}{t}{j}")
                        if last_hb:
                            # final half-block: DVE reciprocal starts right
                            # after ssum instead of waiting for the ACT
                            # table swap — shorter kernel tail
                            nc.vector.reciprocal(r[:], tiles[j][4][:])
                        else:
                            act_recip(r[:], tiles[j][4][:])
                        rr[j] = r

                    for j in (2 * jp, 2 * jp + 1):
                        sf, sg, cp5, so, ssum = tiles[j]
                        a = gp.tile([128, TBLK], f32, tag="a", bufs=3,
                                    name=f"a{l}{t}{j}")
                        nc.gpsimd.tensor_tensor(a[:], sf[:], rr[j][:],
                                                ALU.mult)
                        g = gp.tile([128, TBLK], f32, tag="g", bufs=3,
                                    name=f"g{l}{t}{j}")
                        nc.vector.tensor_tensor(g[:], cp5[:], sg[:], ALU.max)
                        btn = gp.tile([128, TBLK], f32, tag="btn", bufs=2,
                                      name=f"bt{l}{t}{j}")
                        nc.vector.scalar_tensor_tensor(
                            btn[:], a[:], 1.0, g[:], ALU.subtract, ALU.mult)
                        c = cgp.tile([128, TBLK], f32, tag=f"c{j}",
                                     name=f"c{l}{t}{j}")
                        init = cp[l][:, j:j + 1] if t == 0 else carry[l][j]
                        nc.vector.tensor_tensor_scan(c[:], a[:], btn[:],
                                                     init, ALU.mult,
                                                     ALU.subtract)
                        carry[l][j] = c[:, TBLK - 1:TBLK]
                        hdt = fh1 if l == 0 else f32
                        h = gp.tile([128, TBLK], hdt, tag=f"h{l}", bufs=3,
                                    name=f"h{l}{t}{j}")
                        nc.vector.tensor_tensor(h[:], so[:], c[:], ALU.mult)

                        if l == 0:
                            nc.sync.dma_start(
                                h1own[j * 128:(j + 1) * 128, :], h[:])
                        else:
                            nc.sync.dma_start(
                                h2t_d[j * 128:(j + 1) * 128,
                                      t * TBLK:(t + 1) * TBLK], h[:])

                if l == 0:
                    nc.gpsimd.collective_compute(
                        "AllGather", ALU.bypass,
                        replica_groups=[[0, 1], [2, 3], [4, 5], [6, 7]],
                        ins=[h1own.opt()],
                        outs=[h1f[t].opt()],
                    )

    _split_multi_waits(nc)
    return nc


def _shard_inputs(x, W0, b0, W1, b1, c0_prev, c1_prev, mm_mode="bf16"):
    import ml_dtypes
    mmdt = ml_dtypes.bfloat16 if mm_mode == "bf16" else np.float32
    x = np.asarray(x, dtype=np.float32)
    in_maps = []
    xT = [np.ascontiguousarray(x[b].T.astype(mmdt)) for b in range(B)]
    per_layer = []
    for (W, bb) in ((W0, b0), (W1, b1)):
        W = np.asarray(W, dtype=np.float32)
        bb = np.asarray(bb, dtype=np.float32)
        halves = []
        for h in range(2):
            # gate-channel order ct = j*4 + q (chunk-major) so each chunk's
            # four gate weight slices are one contiguous [*, 512] span
            rows = np.concatenate(
                [q * D + h * HALF + j * 128 + np.arange(128)
                 for j in range(4) for q in range(4)])
            wt = np.ascontiguousarray(W[rows, :].T.astype(mmdt))  # (D, GCH)
            ba = np.ascontiguousarray(bb[rows].reshape(16, 128).T)  # (128,16)
            bc = np.ascontiguousarray(
                ba[:, 3::4] + np.float32(0.5))  # cell cols (ct=j*4+3)
            halves.append((wt, ba, bc))
        per_layer.append(halves)
    cps = []
    for cprev in (c0_prev, c1_prev):
        cprev = np.asarray(cprev, dtype=np.float32)
        halves = []
        for b in range(B):
            row = []
            for h in range(2):
                seg = cprev[b, 0, h * HALF:(h + 1) * HALF]
                row.append(np.ascontiguousarray(seg.reshape(4, 128).T))
            halves.append(row)
        cps.append(halves)
    for k in range(NCORES):
        b, h = k // 2, k % 2
        m = {"xT": xT[b]}
        for l in range(2):
            wt, ba, bc = per_layer[l][h]
            m[f"w{l}t"] = wt
            m[f"b{l}a"] = ba
            m[f"b{l}c"] = bc
            m[f"cp{l}"] = cps[l][b][h]
        in_maps.append(m)
    return in_maps


import os
MM_MODE = os.environ.get("MINLSTM_MM_MODE", "bf16")


def _get_nc():
    if "nc" not in _CACHE:
        _CACHE["nc"] = _build_nc(mm_mode=MM_MODE)
    return _CACHE["nc"]


def kernel(x, W0, b0, W1, b1, c0_prev, c1_prev):
    from concourse.bass_utils import run_bass_kernel_spmd

    nc = _get_nc()
    in_maps = _shard_inputs(x, W0, b0, W1, b1, c0_prev, c1_prev, MM_MODE)
    res = run_bass_kernel_spmd(nc, in_maps, list(range(NCORES)))
    out = np.empty((B, S, D), dtype=np.float32)
    for k in range(NCORES):
        b, h = k // 2, k % 2
        out[b, :, h * HALF:(h + 1) * HALF] = res.results[k]["h2t"].T
    return out


# revision 21
# speedup vs baseline: 1.0193x; 1.0193x over previous
"""minLSTM (2-layer, B=4, S=4096, D=1024) on 8 Trainium2 NeuronCores.

Sharding: core k -> (batch b = k//2, channel half h = k%2).
Each core computes all 4096 timesteps for its batch and its 512 channels:
  - gates via PE matmuls in bf16 (lhsT = W^T shard, rhs = x^T), laid out
    (gate-channel partition x token free) so the recurrence layout is native,
  - normalized gates f' = sig(f)/(sig(f)+sig(i)) via ACT sigmoids + one DVE
    reciprocal, with the cheap elementwise ops (ssum, g-max, h-mult) offloaded
    to the otherwise-idle Pool engine,
  - g = max(cell+b+0.5, sig(cell+b)),
  - b~ = (f'-1)*g via one fused scalar_tensor_tensor, then the recurrence
    c_t = f'*c_{t-1} + (1-f')*g_t as tensor_tensor_scan(mult, subtract),
  - h = sig(o) * c.
x-tile loads are prefetched 3 (block,layer) units ahead so they never queue
behind h-stores on the in-order SP DMA queue. Between the two layers,
channel-half pairs exchange h1 (bf16) via pairwise AllGather collectives.

Self-contained: hardcodes shapes; only imports the system concourse repo.
"""
import sys

if '/opt/trn_rl_repo' not in sys.path:
    sys.path.insert(0, '/opt/trn_rl_repo')

import numpy as np

B, S, D = 4, 4096, 1024
NCORES = 8
HALF = D // 2           # channels per core: 512
NCHUNK = HALF // 128    # 4 partition chunks of 128 channels
NKT = D // 128          # 8 contraction k-tiles
TBLK = 512              # token block
NBLK = S // TBLK        # 8 token blocks
GCH = 4 * HALF          # gate channels per core: 2048
PF = 3                  # x-tile prefetch depth in (layer, block) units

_CACHE = {}


def _split_multi_waits(nc):
    """This walrus build rejects >1 sync wait per instruction. Hoist extra
    waits onto same-engine NoOps inserted just before; engine-queue program
    order makes this semantically identical."""
    from concourse import mybir
    n = 0
    for fn in nc.m.functions:
        for blk in fn.blocks:
            insts = list(blk.instructions)
            new = []
            changed = False
            for inst in insts:
                si = inst.sync_info
                ow = list(si.on_wait) if si is not None and si.on_wait else []
                if len(ow) > 1:
                    changed = True
                    for w in ow[:-1]:
                        n += 1
                        nop = mybir.InstNoOp(name=f"I-wsplit-{n}", ins=[], outs=[])
                        nop.engine = inst.engine
                        nop.sync_info = mybir.SyncInfo(on_wait=[w], on_update=[])
                        new.append(nop)
                    si.on_wait = [ow[-1]]
                new.append(inst)
            if changed:
                blk.instructions = new
    return n


def _build_nc(mm_mode="bf16"):
    import concourse.bass as bass
    import concourse.mybir as mybir
    import concourse.tile as tile

    f32 = mybir.dt.float32
    fmm = {"f32r": mybir.dt.float32r, "f32": f32,
           "bf16": mybir.dt.bfloat16}[mm_mode]
    fh1 = mybir.dt.bfloat16 if mm_mode == "bf16" else f32
    AF = mybir.ActivationFunctionType
    ALU = mybir.AluOpType

    nc = bass.Bass("TRN2", target_bir_lowering=False, debug=False,
                   num_devices=NCORES)

    xT_d = nc.dram_tensor("xT", [D, S], fmm, kind="ExternalInput").ap()
    w_d = [nc.dram_tensor(f"w{l}t", [D, GCH], fmm, kind="ExternalInput").ap()
           for l in range(2)]
    ba_d = [nc.dram_tensor(f"b{l}a", [128, 16], f32, kind="ExternalInput").ap()
            for l in range(2)]
    bc_d = [nc.dram_tensor(f"b{l}c", [128, 4], f32, kind="ExternalInput").ap()
            for l in range(2)]
    cp_d = [nc.dram_tensor(f"cp{l}", [128, 4], f32, kind="ExternalInput").ap()
            for l in range(2)]
    h2t_d = nc.dram_tensor("h2t", [HALF, S], f32, kind="ExternalOutput").ap()

    with tile.TileContext(nc) as tc:
        with tc.tile_pool(name="wp", bufs=1) as wp, \
             tc.tile_pool(name="xkp", bufs=PF + 1) as xkp, \
             tc.tile_pool(name="gp", bufs=2) as gp, \
             tc.tile_pool(name="cgp", bufs=3) as cgp, \
             tc.tile_pool(name="cp", bufs=1) as cpool, \
             tc.tile_pool(name="psum", bufs=8, space="PSUM") as psum, \
             tc.tile_pool(name="dstage", bufs=4, space="DRAM") as dstage, \
             tc.tile_pool(name="dfull", bufs=8, space="DRAM") as dfull:

            # h1 gathered blocks must persist through layer 2: 8 live tiles
            h1f = [dfull.tile([D, TBLK], fh1, tag="h1f", name=f"h1f{t}")
                   for t in range(NBLK)]

            # Weight layout (host side): gate-channel index ct = j*4 + q so a
            # chunk j's four gate slices are one contiguous [128,512] span.
            # Layer-0 weights stream per (chunk, k) on the SP queue right
            # behind the first x block; layer-1 weights go on the ACT hwdge
            # queue so they never delay layer-0's pipeline.
            w_ks = {}
            ba = {}
            bc = {}
            cp = {}
            for l in range(2):
                ba[l] = cpool.tile([128, 16], f32, tag=f"ba{l}", name=f"ba{l}")
                bc[l] = cpool.tile([128, 4], f32, tag=f"bc{l}", name=f"bc{l}")
                cp[l] = cpool.tile([128, 4], f32, tag=f"cp{l}", name=f"cp{l}")
                w_ks[l] = [wp.tile([128, GCH], fmm, tag=f"Wk{l}_{k}",
                                   name=f"w{l}_{k}") for k in range(NKT)]

            units = [(l, t) for l in range(2) for t in range(NBLK)]
            xk_tiles = {}

            def load_unit(u):
                l, t = units[u]
                xk_ks = []
                for k in range(NKT):
                    xkt = xkp.tile([128, TBLK], fmm, tag=f"xk{k}",
                                   name=f"xk{l}_{t}_{k}")
                    if l == 0:
                        src = xT_d[k * 128:(k + 1) * 128,
                                   t * TBLK:(t + 1) * TBLK]
                    else:
                        src = h1f[t][k * 128:(k + 1) * 128, :]
                    nc.sync.dma_start(
                        xkt[:],
                        src if src.dtype == fmm else src.bitcast(fmm))
                    xk_ks.append(xkt)
                xk_tiles[u] = xk_ks

            # Startup order: first x block and the weight k-tiles race to
            # feed the first chunk (which consumes k-outer, below); biases
            # and everything else follow. Layer-1 weights ride the ACT hwdge
            # queue so they never delay layer 0.
            nc.sync.dma_start(w_ks[0][0][:], w_d[0][0:128, :])
            load_unit(0)
            for k in range(1, NKT):
                nc.sync.dma_start(w_ks[0][k][:],
                                  w_d[0][k * 128:(k + 1) * 128, :])
            for l in range(2):
                nc.sync.dma_start(ba[l][:], ba_d[l][:])
                nc.sync.dma_start(bc[l][:], bc_d[l][:])
                nc.sync.dma_start(cp[l][:], cp_d[l][:])
            for u in range(1, min(PF, len(units))):
                load_unit(u)
            for k in range(NKT):
                nc.scalar.dma_start(w_ks[1][k][:],
                                    w_d[1][k * 128:(k + 1) * 128, :])

            def act_recip(out_ap, in_ap):
                """ACT-table reciprocal (bass blocks AF.Reciprocal in
                activation(); measured 1.2e-5 max rel err on [9e-5, 2],
                far inside this kernel's tolerance)."""
                eng = nc.scalar
                ins_ = [eng.lower_ap(in_ap)]
                for argv in (0.0, 1.0, 0.0):  # bias, scale, alpha imms
                    ins_.append(mybir.ImmediateValue(dtype=f32, value=argv))
                eng.add_instruction(mybir.InstActivation(
                    name=nc.get_next_instruction_name(),
                    func=AF.Reciprocal,
                    ins=ins_,
                    outs=[eng.lower_ap(out_ap)]))

            carry = {0: [None] * NCHUNK, 1: [None] * NCHUNK}
            for u, (l, t) in enumerate(units):
                if u + PF < len(units):
                    load_unit(u + PF)
                xk_ks = xk_tiles.pop(u)

                if l == 0:
                    h1own = dstage.tile([HALF, TBLK], fh1, tag="h1own",
                                        name=f"h1own{t}")

                def col(j, qi_):
                    return ba[l][:, j * 4 + qi_:j * 4 + qi_ + 1]

                # Per half-block (2 chunks): matmuls + sigmoid-table ACT ops
                # + Pool ssum, then the 2 ACT reciprocals batched so only
                # two table swaps happen per half-block, then the DVE chain.
                # The reciprocal runs on ACT (which has slack), leaving the
                # DVE with only g/btn/scan/h — every engine is now well
                # under the PE's 27.6us/block.
                for jp in range(NCHUNK // 2):
                    tiles = {}
                    for j in (2 * jp, 2 * jp + 1):
                        ps = {}
                        for q in ("i", "f", "o", "cell"):
                            ps[q] = psum.tile([128, TBLK], f32, tag="ps",
                                              name=f"ps_{q}{l}_{t}_{j}")
                        # k-inner (8 MMs per PSUM group) avoids the HAM
                        # psum-cycling throttle; the very first chunk goes
                        # k-outer so the PE starts while weight k-tiles are
                        # still arriving (HAM is cold then anyway).
                        if u == 0 and j == 0:
                            mm_order = [(k, qi) for k in range(NKT)
                                        for qi in range(4)]
                        else:
                            mm_order = [(k, qi) for qi in range(4)
                                        for k in range(NKT)]
                        qnames = ("i", "f", "o", "cell")
                        for k, qi in mm_order:
                            ct = j * 4 + qi
                            nc.tensor.matmul(
                                ps[qnames[qi]][:],
                                w_ks[l][k][:, ct * 128:(ct + 1) * 128],
                                xk_ks[k][:],
                                start=(k == 0), stop=(k == NKT - 1))

                        sf = gp.tile([128, TBLK], f32, tag="sf", bufs=4,
                                     name=f"sf{l}{t}{j}")
                        nc.scalar.activation(sf[:], ps["f"][:], AF.Sigmoid,
                                             bias=col(j, 1))
                        si = gp.tile([128, TBLK], f32, tag="si", bufs=3,
                                     name=f"si{l}{t}{j}")
                        nc.scalar.activation(si[:], ps["i"][:], AF.Sigmoid,
                                             bias=col(j, 0))
                        sg = gp.tile([128, TBLK], f32, tag="sg", bufs=4,
                                     name=f"sg{l}{t}{j}")
                        nc.scalar.activation(sg[:], ps["cell"][:], AF.Sigmoid,
                                             bias=col(j, 3))
                        cp5 = gp.tile([128, TBLK], f32, tag="cp5", bufs=4,
                                      name=f"cq{l}{t}{j}")
                        nc.scalar.activation(cp5[:], ps["cell"][:],
                                             AF.Identity,
                                             bias=bc[l][:, j:j + 1])
                        so = gp.tile([128, TBLK], f32, tag="so", bufs=5,
                                     name=f"so{l}{t}{j}")
                        nc.scalar.activation(so[:], ps["o"][:], AF.Sigmoid,
                                             bias=col(j, 2))
                        ssum = gp.tile([128, TBLK], f32, tag="ssum", bufs=3,
                                       name=f"ss{l}{t}{j}")
                        nc.gpsimd.tensor_tensor(ssum[:], sf[:], si[:],
                                                ALU.add)
                        tiles[j] = (sf, sg, cp5, so, ssum)

                    rr = {}
                    for j in (2 * jp, 2 * jp + 1):
                        r = gp.tile([128, TBLK], f32, tag="r", bufs=3,
                                    name=f"r{l}{t}{j}")
                        act_recip(r[:], tiles[j][4][:])
                        rr[j] = r

                    for j in (2 * jp, 2 * jp + 1):
                        sf, sg, cp5, so, ssum = tiles[j]
                        a = gp.tile([128, TBLK], f32, tag="a", bufs=3,
                                    name=f"a{l}{t}{j}")
                        nc.gpsimd.tensor_tensor(a[:], sf[:], rr[j][:],
                                                ALU.mult)
                        g = gp.tile([128, TBLK], f32, tag="g", bufs=3,
                                    name=f"g{l}{t}{j}")
                        nc.vector.tensor_tensor(g[:], cp5[:], sg[:], ALU.max)
                        btn = gp.tile([128, TBLK], f32, tag="btn", bufs=2,
                                      name=f"bt{l}{t}{j}")
                        nc.vector.scalar_tensor_tensor(
                            btn[:], a[:], 1.0, g[:], ALU.subtract, ALU.mult)
                        c = cgp.tile([128, TBLK], f32, tag=f"c{j}",
                                     name=f"c{l}{t}{j}")
                        init = cp[l][:, j:j + 1] if t == 0 else carry[l][j]
                        nc.vector.tensor_tensor_scan(c[:], a[:], btn[:],
                                                     init, ALU.mult,
                                                     ALU.subtract)
                        carry[l][j] = c[:, TBLK - 1:TBLK]
                        hdt = fh1 if l == 0 else f32
                        h = gp.tile([128, TBLK], hdt, tag=f"h{l}", bufs=3,
                                    name=f"h{l}{t}{j}")
                        nc.vector.tensor_tensor(h[:], so[:], c[:], ALU.mult)

                        if l == 0:
                            nc.sync.dma_start(
                                h1own[j * 128:(j + 1) * 128, :], h[:])
                        else:
                            nc.sync.dma_start(
                                h2t_d[j * 128:(j + 1) * 128,
                                      t * TBLK:(t + 1) * TBLK], h[:])

                if l == 0:
                    nc.gpsimd.collective_compute(
                        "AllGather", ALU.bypass,
                        replica_groups=[[0, 1], [2, 3], [4, 5], [6, 7]],
                        ins=[h1own.opt()],
                        outs=[h1f[t].opt()],
                    )

    _split_multi_waits(nc)
    return nc


def _shard_inputs(x, W0, b0, W1, b1, c0_prev, c1_prev, mm_mode="bf16"):
    import ml_dtypes
    mmdt = ml_dtypes.bfloat16 if mm_mode == "bf16" else np.float32
    x = np.asarray(x, dtype=np.float32)
    in_maps = []
    xT = [np.ascontiguousarray(x[b].T.astype(mmdt)) for b in range(B)]
    per_layer = []
    for (W, bb) in ((W0, b0), (W1, b1)):
        W = np.asarray(W, dtype=np.float32)
        bb = np.asarray(bb, dtype=np.float32)
        halves = []
        for h in range(2):
            # gate-channel order ct = j*4 + q (chunk-major) so each chunk's
            # four gate weight slices are one contiguous [*, 512] span
            rows = np.concatenate(
                [q * D + h * HALF + j * 128 + np.arange(128)
                 for j in range(4) for q in range(4)])
            wt = np.ascontiguousarray(W[rows, :].T.astype(mmdt))  # (D, GCH)
            ba = np.ascontiguousarray(bb[rows].reshape(16, 128).T)  # (128,16)
            bc = np.ascontiguousarray(
                ba[:, 3::4] + np.float32(0.5))  # cell cols (ct=j*4+3)
            halves.append((wt, ba, bc))
        per_layer.append(halves)
    cps = []
    for cprev in (c0_prev, c1_prev):
        cprev = np.asarray(cprev, dtype=np.float32)
        halves = []
        for b in range(B):
            row = []
            for h in range(2):
                seg = cprev[b, 0, h * HALF:(h + 1) * HALF]
                row.append(np.ascontiguousarray(seg.reshape(4, 128).T))
            halves.append(row)
        cps.append(halves)
    for k in range(NCORES):
        b, h = k // 2, k % 2
        m = {"xT": xT[b]}
        for l in range(2):
            wt, ba, bc = per_layer[l][h]
            m[f"w{l}t"] = wt
            m[f"b{l}a"] = ba
            m[f"b{l}c"] = bc
            m[f"cp{l}"] = cps[l][b][h]
        in_maps.append(m)
    return in_maps


import os
MM_MODE = os.environ.get("MINLSTM_MM_MODE", "bf16")


def _get_nc():
    if "nc" not in _CACHE:
        _CACHE["nc"] = _build_nc(mm_mode=MM_MODE)
    return _CACHE["nc"]


def kernel(x, W0, b0, W1, b1, c0_prev, c1_prev):
    from concourse.bass_utils import run_bass_kernel_spmd

    nc = _get_nc()
    in_maps = _shard_inputs(x, W0, b0, W1, b1, c0_prev, c1_prev, MM_MODE)
    res = run_bass_kernel_spmd(nc, in_maps, list(range(NCORES)))
    out = np.empty((B, S, D), dtype=np.float32)
    for k in range(NCORES):
        b, h = k // 2, k % 2
        out[b, :, h * HALF:(h + 1) * HALF] = res.results[k]["h2t"].T
    return out
